# revision 1
# baseline (speedup 1.0000x reference)
"""DeeperGCN (GENConv softmax-aggregation, 28 layers) on 8 Trainium2 NeuronCores.

Sharding: nodes partitioned across the 8 cores balanced by in-degree; each
core owns 2560 node rows (incl. dummy padding) = 20 groups of 128 (nodes
sorted by degree so groups are degree-homogeneous). A node's incoming edges
occupy K_sched[g] slots; edge tensors live in SBUF as
[128 partitions = node-in-group, free = (slot-block, d)].

Per layer: z = relu(LN(carry)) on own nodes -> AllGather z (fp16) into each
core's DRAM -> dma_gather z[src] into slot layout -> per-edge
g = z+ea, r = relu(g), ex = exp(t*r - 8t), p = ex*r  -> segment sums via
TensorE accumulation (identity stationary)  -> aggr = sum_p / max(sum_ex -
npad*e^{-8t}, 1e-9); out = aggr + z + eps -> MLP h1 = out@W1; y =
relu(LN(h1)); carry += y@W2.  The constant C=8 softmax shift is exact
(shift-invariance) and keeps fp16 ranges safe; padded slots contribute
exactly npad*e^{-8t} to the denominator, which is subtracted out.

kernel(**inputs) takes the FULL (unsharded) inputs and returns the FULL
output; sharding/unsharding happens inside.
"""
import os
import sys
import types
import numpy as np

N_CORES = 8
D = 128
H = 256
F = 14
EA = 4
NODES_PER_CORE = 2560
N_GROUPS = NODES_PER_CORE // 128
C_SHIFT = 8.0
EPS = 1e-7
NEG_PAD = -60000.0
CHUNK_BLOCKS = 40  # max slot-blocks per edge-phase SBUF chunk


# ---------------------------------------------------------------------------
# host-side planning
# ---------------------------------------------------------------------------
def _make_plan(edge_index, n_nodes):
    src = np.asarray(edge_index[0]).astype(np.int64)
    dst = np.asarray(edge_index[1]).astype(np.int64)
    deg = np.bincount(dst, minlength=n_nodes)

    order = np.argsort(-deg, kind="stable")
    core_nodes = [[] for _ in range(N_CORES)]
    core_load = np.zeros(N_CORES, dtype=np.int64)
    for i in range(0, n_nodes, N_CORES):
        chunk = order[i:i + N_CORES]
        c_order = np.argsort(core_load, kind="stable")
        for j, nd in enumerate(chunk):
            c = int(c_order[j % N_CORES])
            if len(core_nodes[c]) >= NODES_PER_CORE:
                c = min((cc for cc in range(N_CORES)
                         if len(core_nodes[cc]) < NODES_PER_CORE),
                        key=lambda cc: core_load[cc])
            core_nodes[c].append(nd)
            core_load[c] += deg[nd]

    perm = np.full((N_CORES, NODES_PER_CORE), -1, dtype=np.int64)
    for c in range(N_CORES):
        nd = np.array(core_nodes[c], dtype=np.int64)
        nd = nd[np.argsort(-deg[nd], kind="stable")]
        perm[c, :len(nd)] = nd

    pos = np.full(n_nodes, -1, dtype=np.int64)
    for c in range(N_CORES):
        valid = perm[c] >= 0
        pos[perm[c][valid]] = c * NODES_PER_CORE + np.nonzero(valid)[0]
    assert (pos >= 0).all()

    K = np.zeros((N_CORES, N_GROUPS), dtype=np.int64)
    for c in range(N_CORES):
        for g in range(N_GROUPS):
            nds = perm[c, g * 128:(g + 1) * 128]
            nds = nds[nds >= 0]
            K[c, g] = deg[nds].max() if len(nds) else 0
    K_sched = np.maximum(K.max(axis=0), 1)
    B = int(K_sched.sum())
    group_off = np.concatenate([[0], np.cumsum(K_sched)]).astype(np.int64)

    chunks = []
    g0 = 0
    while g0 < N_GROUPS:
        g1 = g0 + 1
        while g1 < N_GROUPS and (group_off[g1 + 1] - group_off[g0]) <= CHUNK_BLOCKS:
            g1 += 1
        chunks.append((int(g0), int(g1)))
        g0 = g1
    assert all(group_off[b] - group_off[a] <= CHUNK_BLOCKS for a, b in chunks)

    e_order = np.argsort(dst, kind="stable")
    dst_sorted = dst[e_order]
    starts = np.searchsorted(dst_sorted, np.arange(n_nodes))
    ends = np.searchsorted(dst_sorted, np.arange(n_nodes) + 1)

    gather_idx = np.zeros((N_CORES, B * 128), dtype=np.int64)
    slot_edge = np.full((N_CORES, B * 128), -1, dtype=np.int64)
    npad = np.zeros((N_CORES, N_GROUPS, 128), dtype=np.int64)
    for c in range(N_CORES):
        pc = perm[c]
        for g in range(N_GROUPS):
            Kg = int(K_sched[g])
            off = int(group_off[g])
            for p in range(128):
                v = pc[g * 128 + p]
                dv = 0
                if v >= 0:
                    s, e = int(starts[v]), int(ends[v])
                    dv = e - s
                    eids = e_order[s:e]
                    ii = (off + np.arange(dv)) * 128 + p
                    slot_edge[c, ii] = eids
                    gather_idx[c, ii] = pos[src[eids]]
                npad[c, g, p] = Kg - dv

    pl = types.SimpleNamespace()
    pl.n_nodes, pl.deg, pl.perm, pl.pos = n_nodes, deg, perm, pos
    pl.K_sched, pl.B, pl.group_off, pl.chunks = K_sched, B, group_off, chunks
    pl.gather_idx, pl.slot_edge, pl.npad = gather_idx, slot_edge, npad
    return pl


def _wrap_idx(idx_flat):
    """[num] -> [128, num//16] int16, SWDGE wrapped layout: index i lives at
    [i % 16, i // 16], replicated 8x down the partitions."""
    num = idx_flat.shape[0]
    assert num % 16 == 0
    a = np.zeros((16, num // 16), dtype=np.int16)
    a[np.arange(num) % 16, np.arange(num) // 16] = idx_flat.astype(np.int16)
    return np.tile(a, (8, 1))


# ---------------------------------------------------------------------------
# bass program
# ---------------------------------------------------------------------------
def _build(pl, vals, L):
    from contextlib import ExitStack
    import concourse.bass as bass  # noqa: F401
    import concourse.bacc as bacc
    import concourse.mybir as mybir
    import concourse.tile as tile

    f32, f16, i16 = mybir.dt.float32, mybir.dt.float16, mybir.dt.int16
    AF = mybir.ActivationFunctionType
    OP = mybir.AluOpType

    t_host = vals["t"]
    triv_mlp = vals["triv_mlp_affine"]
    triv_b1 = vals["triv_b1"]
    triv_b2 = vals["triv_b2"]
    triv_norm = vals["triv_norm"]
    triv_linb = vals["triv_linb"]

    B = pl.B
    K_sched = [int(k) for k in pl.K_sched]
    goff = [int(o) for o in pl.group_off]
    chunks = pl.chunks
    NT = N_CORES * NODES_PER_CORE

    nc = bacc.Bacc("TRN2", target_bir_lowering=False, debug=False,
                   num_devices=N_CORES)

    gidx_in = nc.dram_tensor("gidx", [128, B * 8], i16, kind="ExternalInput")
    attrT_in = nc.dram_tensor("attrT", [6, B * 128], f16, kind="ExternalInput")
    xT_in = nc.dram_tensor("xT", [16, NODES_PER_CORE], f16, kind="ExternalInput")
    ew6_in = nc.dram_tensor("ew6", [6, D], f16, kind="ExternalInput")
    nw16_in = nc.dram_tensor("nw16", [16, D], f16, kind="ExternalInput")
    ident_in = nc.dram_tensor("ident", [128, 128], f16, kind="ExternalInput")
    W1_in = nc.dram_tensor("W1h", [L, D, H], f16, kind="ExternalInput")
    W2_in = nc.dram_tensor("W2h", [L, H, D], f16, kind="ExternalInput")
    linw_in = nc.dram_tensor("linw", [D, D], f16, kind="ExternalInput")
    npadc_in = nc.dram_tensor("npadc", [128, L * N_GROUPS], f32,
                              kind="ExternalInput")
    cbias_in = nc.dram_tensor("cbias", [128, L + 1], f32, kind="ExternalInput")
    yout = nc.dram_tensor("yout", [NODES_PER_CORE, D], f32, kind="ExternalOutput")
    b1_in = b2_in = linb_in = g1r_in = normr_in = None
    if not all(triv_b1):
        b1_in = nc.dram_tensor("b1r", [L, H], f16, kind="ExternalInput")
    if not all(triv_b2):
        b2_in = nc.dram_tensor("b2r", [L, D], f16, kind="ExternalInput")
    if not triv_linb:
        linb_in = nc.dram_tensor("linbr", [1, D], f16, kind="ExternalInput")
    if not all(triv_mlp):
        g1r_in = nc.dram_tensor("g1r", [L, 2, 128, H], f32, kind="ExternalInput")
    if not all(triv_norm):
        normr_in = nc.dram_tensor("normr", [L, 2, 128, D], f32,
                                  kind="ExternalInput")

    with tile.TileContext(nc) as tc, ExitStack() as es:
        dram = es.enter_context(tc.tile_pool(name="dram", bufs=2, space="DRAM"))
        res = es.enter_context(tc.tile_pool(name="res", bufs=1))
        wk = es.enter_context(tc.tile_pool(name="wk", bufs=2))
        wk1 = es.enter_context(tc.tile_pool(name="wk1", bufs=1))
        sm = es.enter_context(tc.tile_pool(name="sm", bufs=2))
        wpool = es.enter_context(tc.tile_pool(name="wpool", bufs=2))
        pagg = es.enter_context(tc.tile_pool(name="pagg", bufs=3, space="PSUM"))
        pmm = es.enter_context(tc.tile_pool(name="pmm", bufs=2, space="PSUM"))
        pmm2 = es.enter_context(tc.tile_pool(name="pmm2", bufs=2, space="PSUM"))

        # residents
        ea = res.tile([128, B, D], f16, name="ea", tag="ea")
        carry = res.tile([128, N_GROUPS, D], f32, name="carry", tag="carry")
        z_own = res.tile([128, N_GROUPS, D], f16, name="z_own", tag="z_own")
        outt = res.tile([128, N_GROUPS, D], f16, name="outt", tag="outt")
        h1 = res.tile([128, N_GROUPS, H], f16, name="h1", tag="h1")
        gidx = res.tile([128, B * 8], i16, name="gidx", tag="gidx")
        npadc = res.tile([128, L * N_GROUPS], f32, name="npadc", tag="npadc")
        cbias = res.tile([128, L + 1], f32, name="cbias", tag="cbias")
        ident = res.tile([128, 128], f16, name="ident", tag="ident")
        ones1 = res.tile([1, 128], f16, name="ones1", tag="ones1")
        ew6 = res.tile([6, D], f16, name="ew6", tag="ew6")
        nw16 = res.tile([16, D], f16, name="nw16", tag="nw16")
        xT = res.tile([16, NODES_PER_CORE], f16, name="xT", tag="xT")
        linw = res.tile([D, D], f16, name="linw", tag="linw")

        nc.sync.dma_start(gidx[:], gidx_in.ap())
        nc.sync.dma_start(npadc[:], npadc_in.ap())
        nc.sync.dma_start(cbias[:], cbias_in.ap())
        nc.sync.dma_start(ident[:], ident_in.ap())
        nc.sync.dma_start(ew6[:], ew6_in.ap())
        nc.sync.dma_start(nw16[:], nw16_in.ap())
        nc.sync.dma_start(xT[:], xT_in.ap())
        nc.sync.dma_start(linw[:], linw_in.ap())
        nc.vector.memset(ones1[:], 1.0)

        # ---- ea_slot init -------------------------------------------------
        for (g0, g1) in chunks:
            b0, b1_ = goff[g0], goff[g1]
            sc = b1_ - b0
            at = wk.tile([6, sc * 128], f16, name="at", tag="zg")
            nc.sync.dma_start(at[:], attrT_in.ap()[:, b0 * 128:b1_ * 128])
            for b in range(sc):
                pe_ea = pmm2.tile([128, D], f32, name="pe_ea", tag="pmm2")
                nc.tensor.matmul(pe_ea[:], at[:, b * 128:(b + 1) * 128], ew6[:],
                                 start=True, stop=True)
                nc.scalar.copy(ea[:, b0 + b, :], pe_ea[:])

        # ---- node encoder -> z_own (layer-0 "z") --------------------------
        for t in range(N_GROUPS):
            pe_enc = pmm2.tile([128, D], f32, name="pe_enc", tag="pmm2")
            nc.tensor.matmul(pe_enc[:], xT[:, t * 128:(t + 1) * 128], nw16[:],
                             start=True, stop=True)
            nc.scalar.copy(z_own[:, t, :], pe_enc[:])

        DBG_NO_COLL = os.environ.get("DEBUG_NO_COLLECTIVE", "0") == "1"
        DBG_NO_GATHER = os.environ.get("DEBUG_NO_GATHER", "0") == "1"
        DBG_NO_XPOSE = os.environ.get("DEBUG_NO_XPOSE", "0") == "1"

        def broadcast_z():
            zb_t = dram.tile([NODES_PER_CORE * D], f16, name="zb", tag="zb")
            zd_t = dram.tile([NT, D], f16, name="zd", tag="zd",
                             addr_space="Local" if os.environ.get("DEBUG_ZD_LOCAL", "0") == "1" else "Shared")
            nc.sync.dma_start(
                zb_t[:].rearrange("(g p d) -> p g d", p=128, d=D), z_own[:])
            if DBG_NO_COLL:
                nc.sync.dma_start(zd_t[:].rearrange("(n d) -> n d", d=D)[0:NODES_PER_CORE, :]
                                  if False else zd_t[0:NODES_PER_CORE, :],
                                  zb_t[:].rearrange("(n d) -> n d", d=D))
            else:
                nc.gpsimd.collective_compute(
                    "AllGather", OP.bypass,
                    replica_groups=[list(range(N_CORES))],
                    ins=[zb_t.opt()], outs=[zd_t.opt()])
            return zd_t

        def ln_apply(src_tile, width, dst, dst_slice, trivial, rep_in, rep_idx):
            """LN over free dim (width) of src_tile[:, t, :] for all t, then
            affine+relu into dst_slice(dst, t) (fp16 out)."""
            stats = sm.tile([128, N_GROUPS, 6], f32, name="stats",
                            tag=f"stats{width}")
            mv = sm.tile([128, N_GROUPS, 2], f32, name="mv", tag=f"mv{width}")
            for t in range(N_GROUPS):
                nc.vector.bn_stats(stats[:, t, :], src_tile[:, t, :])
                nc.vector.bn_aggr(mv[:, t, :], stats[:, t, :])
            std = sm.tile([128, N_GROUPS], f32, name="std", tag="std")
            rstd = sm.tile([128, N_GROUPS], f32, name="rstd", tag="rstd")
            nmrs = sm.tile([128, N_GROUPS], f32, name="nmrs", tag="nmrs")
            nc.scalar.activation(std[:], mv[:, :, 1], AF.Sqrt,
                                 bias=cbias[:, L:L + 1])
            nc.vector.reciprocal(rstd[:], std[:])
            nc.vector.tensor_tensor(nmrs[:], mv[:, :, 0], rstd[:], OP.mult)
            nc.vector.tensor_scalar_mul(nmrs[:], nmrs[:], -1.0)
            if trivial:
                for t in range(N_GROUPS):
                    nc.scalar.activation(dst_slice(dst, t), src_tile[:, t, :],
                                         AF.Relu, bias=nmrs[:, t:t + 1],
                                         scale=rstd[:, t:t + 1])
            else:
                gg = wk.tile([128, width], f32, name="gg", tag=f"affg{width}")
                bb = wk.tile([128, width], f32, name="bb", tag=f"affb{width}")
                nc.sync.dma_start(gg[:], rep_in.ap()[rep_idx, 0])
                nc.sync.dma_start(bb[:], rep_in.ap()[rep_idx, 1])
                for t in range(N_GROUPS):
                    tmp = wk.tile([128, width], f32, name="tmp",
                                  tag=f"afft{width}")
                    nc.scalar.activation(tmp[:], src_tile[:, t, :], AF.Identity,
                                         bias=nmrs[:, t:t + 1],
                                         scale=rstd[:, t:t + 1])
                    nc.vector.tensor_tensor(tmp[:], tmp[:], gg[:], OP.mult)
                    nc.vector.tensor_tensor(tmp[:], tmp[:], bb[:], OP.add)
                    nc.vector.tensor_scalar_max(dst_slice(dst, t), tmp[:], 0.0)

        in_group_slice = lambda dstt, t: dstt[:, t, :]

        # ---- layers -------------------------------------------------------
        for l in range(L):
            tl = float(t_host[l])
            zd_t = broadcast_z()

            W1t = wpool.tile([D, H], f16, name="W1t", tag="W1")
            W2a = wpool.tile([128, D], f16, name="W2a", tag="W2a")
            W2b = wpool.tile([128, D], f16, name="W2b", tag="W2b")
            nc.sync.dma_start(W1t[:], W1_in.ap()[l])
            nc.sync.dma_start(W2a[:], W2_in.ap()[l, 0:128, :])
            nc.sync.dma_start(W2b[:], W2_in.ap()[l, 128:256, :])
            b1row = b2row = None
            if b1_in is not None and not triv_b1[l]:
                b1row = wpool.tile([1, H], f16, name="b1row", tag="b1row")
                nc.sync.dma_start(b1row[:], b1_in.ap()[l:l + 1, :])
            if b2_in is not None and not triv_b2[l]:
                b2row = wpool.tile([1, D], f16, name="b2row", tag="b2row")
                nc.sync.dma_start(b2row[:], b2_in.ap()[l:l + 1, :])

            # ---- edge phase ----
            for (g0, g1) in chunks:
                b0, b1_ = goff[g0], goff[g1]
                sc = b1_ - b0
                zg = wk.tile([128, sc, D], f16, name="zg", tag="zg")
                expp = wk1.tile([128, sc, 2, D], f16, name="expp", tag="expp")
                if DBG_NO_GATHER:
                    nc.vector.memset(zg[:], 0.25)
                else:
                    nc.gpsimd.dma_gather(zg[:], zd_t[:], gidx[:, b0 * 8:b1_ * 8],
                                         sc * 128, sc * 128, D,
                                         single_packet=False)
                nc.vector.tensor_tensor(zg[:], zg[:], ea[:, b0:b1_, :], OP.add)
                nc.vector.tensor_scalar_max(zg[:], zg[:], 0.0)
                nc.scalar.activation(expp[:, :, 0, :], zg[:], AF.Exp,
                                     bias=cbias[:, l:l + 1], scale=tl)
                nc.vector.tensor_tensor(expp[:, :, 1, :], expp[:, :, 0, :],
                                        zg[:], OP.mult)
                for g in range(g0, g1):
                    Kg = K_sched[g]
                    off = goff[g] - b0
                    ps = pagg.tile([128, 2 * D], f32, name="ps", tag="agg")
                    for k in range(Kg):
                        nc.tensor.matmul(
                            ps[:], ident[:],
                            expp[:, off + k, :, :].rearrange("p a b -> p (a b)"),
                            start=(k == 0), stop=(k == Kg - 1))
                    S = sm.tile([128, D], f32, name="S", tag="S")
                    rcp = sm.tile([128, D], f32, name="rcp", tag="rcp")
                    scr = sm.tile([128, D], f32, name="scr", tag="scr")
                    nc.vector.tensor_scalar(
                        S[:], ps[:, 0:D],
                        npadc[:, l * N_GROUPS + g:l * N_GROUPS + g + 1],
                        1e-9, OP.subtract, OP.max)
                    nc.vector.reciprocal_approx_accurate(rcp[:], S[:], scr[:])
                    arg = sm.tile([128, D], f16, name="arg", tag="arg")
                    nc.vector.tensor_tensor(arg[:], ps[:, D:2 * D], rcp[:],
                                            OP.mult)
                    nc.vector.tensor_tensor(outt[:, g, :], arg[:],
                                            z_own[:, g, :], OP.add)
                    nc.vector.tensor_scalar_add(outt[:, g, :], outt[:, g, :],
                                                EPS)

            # ---- MLP ----
            for t in range(N_GROUPS):
                oT = sm.tile([128, 128], f16, name="oT", tag="oT")
                if DBG_NO_XPOSE:
                    nc.vector.tensor_copy(oT[:], outt[:, t, :])
                else:
                    nc.sync.dma_start_transpose(oT[:], outt[:, t, :])
                ph1 = pmm.tile([128, H], f32, name="ph1", tag="mm1")
                nc.tensor.matmul(ph1[:], oT[:], W1t[:],
                                 start=True, stop=(b1row is None))
                if b1row is not None:
                    nc.tensor.matmul(ph1[:], ones1[:], b1row[:],
                                     start=False, stop=True)
                nc.scalar.copy(h1[:, t, :], ph1[:])
            ln_apply(h1, H, h1, in_group_slice, triv_mlp[l], g1r_in, l)
            for t in range(N_GROUPS):
                yTa = sm.tile([128, 128], f16, name="yTa", tag="yTa")
                yTb = sm.tile([128, 128], f16, name="yTb", tag="yTb")
                if DBG_NO_XPOSE:
                    nc.vector.tensor_copy(yTa[:], h1[:, t, 0:128])
                    nc.vector.tensor_copy(yTb[:], h1[:, t, 128:256])
                else:
                    nc.sync.dma_start_transpose(yTa[:], h1[:, t, 0:128])
                    nc.sync.dma_start_transpose(yTb[:], h1[:, t, 128:256])
                po = pmm2.tile([128, D], f32, name="po", tag="pmm2")
                nc.tensor.matmul(po[:], yTa[:], W2a[:], start=True, stop=False)
                nc.tensor.matmul(po[:], yTb[:], W2b[:],
                                 start=False, stop=(b2row is None))
                if b2row is not None:
                    nc.tensor.matmul(po[:], ones1[:], b2row[:],
                                     start=False, stop=True)
                if l == 0:
                    nc.vector.tensor_copy(carry[:, t, :], po[:])
                else:
                    nc.vector.tensor_tensor(carry[:, t, :], carry[:, t, :],
                                            po[:], OP.add)

            if l + 1 < L:
                ln_apply(carry, D, z_own, in_group_slice, triv_norm[l + 1],
                         normr_in, l + 1)

        # ---- final head ----
        ln_apply(carry, D, z_own, in_group_slice, triv_norm[0], normr_in, 0)
        lbrow = None
        if linb_in is not None:
            lbrow = res.tile([1, D], f16, name="lbrow", tag="lbrow")
            nc.sync.dma_start(lbrow[:], linb_in.ap())
        for t in range(N_GROUPS):
            zT = sm.tile([128, 128], f16, name="zT", tag="oT")
            if DBG_NO_XPOSE:
                nc.vector.tensor_copy(zT[:], z_own[:, t, :])
            else:
                nc.sync.dma_start_transpose(zT[:], z_own[:, t, :])
            py = pmm2.tile([128, D], f32, name="py", tag="pmm2")
            nc.tensor.matmul(py[:], zT[:], linw[:], start=True,
                             stop=(lbrow is None))
            if lbrow is not None:
                nc.tensor.matmul(py[:], ones1[:], lbrow[:], start=False,
                                 stop=True)
            ysb = sm.tile([128, D], f32, name="ysb", tag="ysb")
            nc.vector.tensor_copy(ysb[:], py[:])
            nc.sync.dma_start(
                yout.ap().rearrange("(g p) d -> p g d", p=128)[:, t, :], ysb[:])

    nc.compile()
    return nc


# ---------------------------------------------------------------------------
# public entry point
# ---------------------------------------------------------------------------
def kernel(**inputs):
    inputs = {k: np.asarray(v) for k, v in inputs.items()}
    x = inputs["x"].astype(np.float32)
    edge_attr = inputs["edge_attr"].astype(np.float32)
    edge_index = inputs["edge_index"]
    n_nodes = x.shape[0]
    L = int(inputs["W1"].shape[0])
    L_run = int(os.environ.get("KERNEL_LAYERS", L))

    pl = _make_plan(edge_index, n_nodes)

    t = inputs["t"].astype(np.float64)
    assert (t > 0).all(), "kernel assumes positive softmax temperature"
    W1 = inputs["W1"].astype(np.float32)
    b1 = inputs["b1"].astype(np.float32)
    g1 = inputs["g1"].astype(np.float32)
    be1 = inputs["be1"].astype(np.float32)
    W2 = inputs["W2"].astype(np.float32)
    b2 = inputs["b2"].astype(np.float32)
    norm_g = inputs["norm_g"].astype(np.float32)
    norm_b = inputs["norm_b"].astype(np.float32)
    lin_w = inputs["lin_w"].astype(np.float32)
    lin_b = inputs["lin_b"].astype(np.float32)

    vals = {
        "t": [float(v) for v in t[:L_run]],
        "triv_b1": [bool((b1[l] == 0).all()) for l in range(L_run)],
        "triv_b2": [bool((b2[l] == 0).all()) for l in range(L_run)],
        "triv_mlp_affine": [bool((g1[l] == 1).all() and (be1[l] == 0).all())
                            for l in range(L_run)],
        "triv_norm": [bool((norm_g[l] == 1).all() and (norm_b[l] == 0).all())
                      for l in range(L_run)],
        "triv_linb": bool((lin_b == 0).all()),
    }

    nc = _build(pl, vals, L_run)

    B = pl.B
    ew6 = np.zeros((6, D), np.float16)
    ew6[:EA] = inputs["edge_w"].astype(np.float16)
    ew6[EA] = inputs["edge_b"].astype(np.float16)
    ew6[EA + 1] = NEG_PAD
    nw16 = np.zeros((16, D), np.float16)
    nw16[:F] = inputs["node_w"].astype(np.float16)
    nw16[F] = inputs["node_b"].astype(np.float16)
    identm = np.eye(128, dtype=np.float16)
    W1h = np.ascontiguousarray(W1[:L_run].astype(np.float16))
    W2h = np.ascontiguousarray(W2[:L_run].astype(np.float16))
    linwh = lin_w.astype(np.float16)
    exp_shift = np.exp(-C_SHIFT * t[:L_run]).astype(np.float32)
    cbias_host = np.zeros((128, L_run + 1), np.float32)
    cbias_host[:, :L_run] = (-C_SHIFT * t[:L_run]).astype(np.float32)[None, :]
    cbias_host[:, L_run] = 1e-5

    in_maps = []
    for c in range(N_CORES):
        attrT = np.zeros((6, B * 128), np.float16)
        se = pl.slot_edge[c]
        valid = se >= 0
        attrT[:EA, valid] = edge_attr[se[valid]].T.astype(np.float16)
        attrT[EA, valid] = 1.0
        attrT[EA + 1, ~valid] = 1.0
        xT = np.zeros((16, NODES_PER_CORE), np.float16)
        pc = pl.perm[c]
        validn = pc >= 0
        xT[:F, validn] = x[pc[validn]].T.astype(np.float16)
        xT[F, validn] = 1.0
        npadc = np.zeros((128, L_run * N_GROUPS), np.float32)
        for l in range(L_run):
            for g in range(N_GROUPS):
                npadc[:, l * N_GROUPS + g] = pl.npad[c, g] * exp_shift[l]
        m = {
            "gidx": _wrap_idx(pl.gather_idx[c]),
            "attrT": attrT, "xT": xT, "ew6": ew6, "nw16": nw16,
            "ident": identm, "W1h": W1h, "W2h": W2h, "linw": linwh,
            "npadc": npadc, "cbias": cbias_host,
        }
        if not all(vals["triv_b1"]):
            m["b1r"] = b1[:L_run].astype(np.float16)
        if not all(vals["triv_b2"]):
            m["b2r"] = b2[:L_run].astype(np.float16)
        if not vals["triv_linb"]:
            m["linbr"] = lin_b[None].astype(np.float16)
        if not all(vals["triv_mlp_affine"]):
            m["g1r"] = np.broadcast_to(
                np.stack([g1[:L_run], be1[:L_run]], 1)[:, :, None, :],
                (L_run, 2, 128, H)).astype(np.float32).copy()
        if not all(vals["triv_norm"]):
            m["normr"] = np.broadcast_to(
                np.stack([norm_g[:L_run], norm_b[:L_run]], 1)[:, :, None, :],
                (L_run, 2, 128, D)).astype(np.float32).copy()
        in_maps.append(m)

    from concourse import bass_utils
    trace = os.environ.get("KERNEL_TRACE", "0") == "1"
    if trace:
        _install_ntff_shim()
    res = bass_utils.run_bass_kernel_spmd(
        nc, in_maps, core_ids=list(range(N_CORES)), trace=trace)
    if trace and res.exec_time_ns is not None:
        print(f"HW exec time: {res.exec_time_ns} ns")
        kernel.last_exec_time_ns = res.exec_time_ns
    kernel.last_results = res

    out_full = np.zeros((n_nodes, D), np.float32)
    for c in range(N_CORES):
        y = res.results[c]["yout"]
        pc = pl.perm[c]
        validn = pc >= 0
        out_full[pc[validn]] = y[np.nonzero(validn)[0]]
    return out_full


def _install_ntff_shim():
    import antenv
    if hasattr(antenv, "axon_hooks"):
        return
    mod = types.ModuleType("antenv.axon_hooks")
    _HOOK = [None]
    mod.set_axon_ntff_profile_hook = lambda h: _HOOK.__setitem__(0, h)
    mod.get_axon_ntff_profile_hook = lambda: _HOOK[0]
    sys.modules["antenv.axon_hooks"] = mod
    antenv.axon_hooks = mod
    if "/root/.axon_site" not in sys.path:
        sys.path.insert(0, "/root/.axon_site")
    from trn_agent_boot.trn_boot import _ntff_profile_via_ctypes
    mod.set_axon_ntff_profile_hook(
        _ntff_profile_via_ctypes("/opt/axon/libaxon_pjrt.so"))

